# revision 3
# baseline (speedup 1.0000x reference)
"""BFP activation quantization kernel for Trainium2 (8 NeuronCores).

Problem: x (64, 256, 56, 56) fp32. Channels grouped in blocks of 32; each
block shares the max frexp-exponent emax; mantissas truncated to
`mantissa_bits` bits relative to 2^emax:
    q = trunc(x / 2^(emax-mb)) * 2^(emax-mb)

Pipeline (all identities bit-exact, verified on hardware):
  - M = max_c |x| per (block, pixel); Pt = bits(M) & 0x7F800000 = 2^(emax-1).
  - SC = 2^(emax-mb) (kept in bf16: exact, power of two); INV = 2^(mb-emax)
    built by integer exponent arithmetic on Pt (exact reciprocal).
  - ya = x * INV (signed; exact power-of-two scaling), |ya| in [0, 2^mb).
  - ACT engine: A = |ya|; the fp32->int16 convert rounds to nearest-even, so
    trunc is built from two rounded candidates:
        t16  = rne16(A - 0.5)        (ACT, bias=-0.5)
        nu16 = rne16(-A - 0.5) = -u16(ACT, scale=-1, bias=-0.5)
        u16m1 = -nu16 - 1            (DVE int16 ts: mult -1, add -1)
        tr   = max(u16m1, t16)       (DVE int16 TT)
    which equals trunc(|ya|) for every case incl. integer ya and half ties.
  - q = (tr * SC) * Sign(x): tr <= 2^mb - 1 has <= 8 significant bits, so for
    mb <= 8 every product is exactly representable in bf16; Sign(x) comes
    from the ACT engine as bf16 +-1 (0 for x == 0, where q = 0 anyway).
  - Output is stored as bf16 (exact) and widened to fp32 on the host.

Engine split per tile (1 image; partition p = (b<8, g<16), free = (c32,s196)):
  DVE: reduce + 3 small [P,196] ts + ya-mult + int16/bf16 tail (4 ops)
  ACT: Sign, Abs, t16, nu16 (own port; overlaps DVE fully)
  SP:  8 load + 8 store DMAs per tile, double-buffered
(GPSIMD/Pool cannot run TensorTensor/TensorScalar in this toolchain; PE's
fp32r matmuls are not bit-exact - both verified - so DVE+ACT is the split.)

Sharding: data-parallel on N across 8 cores, no cross-core communication.
"""

import numpy as np

N_CORES = 8
N, C, H, W = 64, 256, 56, 56
HW = H * W                   # 3136
N_PER_CORE = N // N_CORES    # 8
B = 8                        # channel blocks
SIG = 16                     # spatial chunks per image
C_IN = 32                    # channels per block
S = HW // SIG                # 196
P = B * SIG                  # 128 partitions
TILES = N_PER_CORE           # 8 (one image per tile)
DMAS = B                     # dma_starts per tile per direction
INC = 16 * DMAS              # load-sem increment per tile (128)

TRACE = False
LAST_RESULTS = None
_CACHE = {}


def _build(mbits: int):
    import concourse.bass as bass
    from concourse import mybir

    nc = bass.Bass()
    x_in = nc.declare_dram_parameter(
        "x", [N_PER_CORE, C, HW], mybir.dt.float32, isOutput=False
    )
    q_out = nc.declare_dram_parameter(
        "q", [N_PER_CORE, C, HW], mybir.dt.bfloat16, isOutput=True
    )
    src = x_in[:].rearrange("n (b c) (g s) -> n b g c s", c=C_IN, s=S)
    dst = q_out[:].rearrange("n (b c) (g s) -> n b g c s", c=C_IN, s=S)

    i32, f32, i16, bf16 = (
        mybir.dt.int32, mybir.dt.float32, mybir.dt.int16, mybir.dt.bfloat16
    )
    Alu = mybir.AluOpType
    Act = mybir.ActivationFunctionType

    from contextlib import ExitStack
    es = ExitStack()
    with es:
        sb = lambda nm, shape, dt: es.enter_context(nc.sbuf_tensor(nm, shape, dt))
        X0 = sb("X0", [P, C_IN, S], f32); X1 = sb("X1", [P, C_IN, S], f32)
        YA0 = sb("YA0", [P, C_IN, S], f32); YA1 = sb("YA1", [P, C_IN, S], f32)
        T16 = sb("T16", [P, C_IN, S], i16)
        NU16 = sb("NU16", [P, C_IN, S], i16)
        QB0 = sb("QB0", [P, C_IN, S], bf16); QB1 = sb("QB1", [P, C_IN, S], bf16)
        SG0 = sb("SG0", [P, C_IN, S], bf16); SG1 = sb("SG1", [P, C_IN, S], bf16)
        M = sb("Mt", [P, S], f32)
        Pt = sb("Ptt", [P, S], i32)
        INV = sb("INVt", [P, S], f32)
        SC0 = sb("SC0", [P, S], bf16); SC1 = sb("SC1", [P, S], bf16)
        load_sem = es.enter_context(nc.semaphore())
        store_sem = es.enter_context(nc.semaphore())
        dve_sem = es.enter_context(nc.semaphore())
        act_sem = es.enter_context(nc.semaphore())
        block = es.enter_context(nc.Block())
        X = [X0, X1]
        YA = [YA0, YA1]
        QB = [QB0, QB1]
        SG = [SG0, SG1]
        SC = [SC0, SC1]
        y_done = {}    # dve counter after ya(t)
        tr_done = {}   # dve counter after tr(t)
        s_done = {}    # dve counter after final sign mult(t)

        def bc(ap):
            return ap.unsqueeze(1).broadcast_to((P, C_IN, S))

        @block.vector
        def _(vector):
            k = 0

            def step(inst):
                nonlocal k
                inst.then_inc(dve_sem, 1)
                k += 1
                vector.wait_ge(dve_sem, k)

            def front(t):
                nonlocal k
                xb = X[t % 2]
                vector.wait_ge(load_sem, INC * (t + 1))
                if t >= 2:
                    # YA[t%2] is free once ACT's nu16(t-2) is done
                    vector.wait_ge(act_sem, 4 * (t - 2) + 4)
                step(vector.tensor_reduce(
                    out=M[:], in_=xb[:].rearrange("p c s -> p s c"),
                    axis=mybir.AxisListType.X, op=Alu.max,
                    apply_absolute_value=True,
                ))
                step(vector.tensor_scalar(
                    out=Pt[:], in0=M[:].bitcast(i32),
                    scalar1=0x7F800000, scalar2=None, op0=Alu.bitwise_and,
                ))
                step(vector.tensor_scalar(
                    out=SC[t % 2][:], in0=Pt[:].bitcast(f32),
                    scalar1=float(2.0 ** (1 - mbits)), scalar2=float(2.0 ** -126),
                    op0=Alu.mult, op1=Alu.max,
                ))
                step(vector.tensor_scalar(
                    out=INV[:].bitcast(i32), in0=Pt[:],
                    scalar1=(253 + mbits) << 23, scalar2=-1,
                    op0=Alu.subtract, op1=Alu.mult,
                ))
                step(vector.tensor_tensor(
                    out=YA[t % 2][:], in0=xb[:], in1=bc(INV[:]), op=Alu.mult,
                ))
                y_done[t] = k

            def back(t):
                nonlocal k
                vector.wait_ge(act_sem, 4 * t + 4)  # ACT(t) fully done
                if t >= 2:
                    vector.wait_ge(store_sem, INC * (t - 1))  # QB[t%2] free
                # u16m1 = -nu16 - 1 (in-place)
                step(vector.tensor_scalar(
                    out=NU16[:], in0=NU16[:],
                    scalar1=-1, scalar2=-1, op0=Alu.mult, op1=Alu.add,
                ))
                # tr = max(u16m1, t16) (in-place into T16)
                step(vector.tensor_tensor(
                    out=T16[:], in0=NU16[:], in1=T16[:], op=Alu.max,
                ))
                tr_done[t] = k
                # qpos = tr * SC -> bf16
                step(vector.tensor_tensor(
                    out=QB[t % 2][:], in0=T16[:], in1=bc(SC[t % 2][:]),
                    op=Alu.mult,
                ))
                # q = qpos * sign(x)
                step(vector.tensor_tensor(
                    out=QB[t % 2][:], in0=QB[t % 2][:], in1=SG[t % 2][:],
                    op=Alu.mult,
                ))
                s_done[t] = k

            front(0)
            for t in range(1, TILES):
                front(t)
                back(t - 1)
            back(TILES - 1)

        @block.scalar
        def _(scalar):
            for t in range(TILES):
                # sgn(t) reads X[t%2], writes SG[t%2] (read by S(t-2) on DVE)
                if t >= 2:
                    scalar.wait_ge(dve_sem, s_done[t - 2])
                scalar.wait_ge(load_sem, INC * (t + 1))
                scalar.activation(
                    out=SG[t % 2][:], in_=X[t % 2][:],
                    func=Act.Sign, bias=0.0, scale=1.0,
                ).then_inc(act_sem, 1)
                # A = |ya| in-place; needs DVE ya(t)
                scalar.wait_ge(dve_sem, y_done[t])
                scalar.activation(
                    out=YA[t % 2][:], in_=YA[t % 2][:],
                    func=Act.Abs, bias=0.0, scale=1.0,
                ).then_inc(act_sem, 1)
                # t16/nu16 overwrite buffers consumed by DVE back(t-1)
                if t >= 1:
                    scalar.wait_ge(dve_sem, tr_done[t - 1])
                scalar.activation(
                    out=T16[:], in_=YA[t % 2][:],
                    func=Act.Copy, bias=-0.5, scale=1.0,
                ).then_inc(act_sem, 1)
                scalar.activation(
                    out=NU16[:], in_=YA[t % 2][:],
                    func=Act.Copy, bias=-0.5, scale=-1.0,
                ).then_inc(act_sem, 1)

        def issue_loads(sync, t):
            xb = X[t % 2]
            for b in range(B):
                sync.dma_start(
                    out=xb[b * SIG:(b + 1) * SIG], in_=src[t, b]
                ).then_inc(load_sem, 16)

        @block.sync
        def _(sync):
            issue_loads(sync, 0)
            sync.wait_ge(load_sem, INC)
            issue_loads(sync, 1)
            for t in range(TILES):
                sync.wait_ge(dve_sem, s_done[t])
                qb = QB[t % 2]
                for b in range(B):
                    sync.dma_start(
                        out=dst[t, b], in_=qb[b * SIG:(b + 1) * SIG]
                    ).then_inc(store_sem, 16)
                if t + 2 < TILES:
                    # X[t%2] free once ACT consumed it for sgn(t)
                    sync.wait_ge(act_sem, 4 * t + 1)
                    sync.wait_ge(load_sem, INC * (t + 2))
                    issue_loads(sync, t + 2)

    return nc


def kernel(activations, mantissa_bits, blk, **_ignored):
    global LAST_RESULTS
    from concourse.bass_utils import run_bass_kernel_spmd

    mbits = int(mantissa_bits)
    assert int(blk) == C_IN, f"kernel hardcodes blk=32, got {blk}"
    assert 1 <= mbits <= 8, f"bf16 output path requires mantissa_bits<=8, got {mbits}"
    x = np.ascontiguousarray(np.asarray(activations), dtype=np.float32)
    assert x.shape == (N, C, H, W), x.shape

    if mbits not in _CACHE:
        _CACHE[mbits] = _build(mbits)
    nc = _CACHE[mbits]

    shards = x.reshape(N_CORES, N_PER_CORE, C, HW)
    in_maps = [{"x": shards[i]} for i in range(N_CORES)]
    res = run_bass_kernel_spmd(nc, in_maps, list(range(N_CORES)), trace=TRACE)
    LAST_RESULTS = res
    out = np.stack(
        [res.results[i]["q"].astype(np.float32) for i in range(N_CORES)], axis=0
    )
    return out.reshape(N, C, H, W)


# revision 4
# speedup vs baseline: 1.4843x; 1.4843x over previous
"""BFP activation quantization kernel for Trainium2 (8 NeuronCores).

Problem: x (64, 256, 56, 56) fp32. Channels grouped in blocks of 32; each
block shares the max frexp-exponent emax; mantissas truncated to
`mantissa_bits` bits relative to 2^emax:
    q = trunc(x / 2^(emax-mb)) * 2^(emax-mb)

Pipeline (all identities bit-exact, verified on hardware):
  - M = max_c |x| per (block, pixel); Pt = bits(M) & 0x7F800000 = 2^(emax-1).
  - SC = 2^(emax-mb) (kept in bf16: exact, power of two); INV = 2^(mb-emax)
    built by integer exponent arithmetic on Pt (exact reciprocal).
  - ya = x * INV (signed; exact power-of-two scaling), |ya| in [0, 2^mb).
  - ACT engine: A = |ya|; the fp32->int16 convert rounds to nearest-even, so
    trunc is built from two rounded candidates:
        t16  = rne16(A - 0.5)        (ACT, bias=-0.5)
        nu16 = rne16(-A - 0.5) = -u16(ACT, scale=-1, bias=-0.5)
        u16m1 = -nu16 - 1            (DVE int16 ts: mult -1, add -1)
        tr   = max(u16m1, t16)       (DVE int16 TT)
    which equals trunc(|ya|) for every case incl. integer ya and half ties.
  - q = (tr * SC) * Sign(x): tr <= 2^mb - 1 has <= 8 significant bits, so for
    mb <= 8 every product is exactly representable in bf16; Sign(x) comes
    from the ACT engine as bf16 +-1 (0 for x == 0, where q = 0 anyway).
  - Output is stored as bf16 (exact) and widened to fp32 on the host.

Engine split per tile (1 image; partition p = (b<8, g<16), free = (c32,s196)):
  DVE: reduce + 3 small [P,196] ts + ya-mult + int16/bf16 tail (4 ops)
  ACT: Sign, Abs, t16, nu16 (own port; overlaps DVE fully)
  SP:  8 load + 8 store DMAs per tile, double-buffered
(GPSIMD/Pool cannot run TensorTensor/TensorScalar in this toolchain; PE's
fp32r matmuls are not bit-exact - both verified - so DVE+ACT is the split.)

Sharding: data-parallel on N across 8 cores, no cross-core communication.
"""

import numpy as np

N_CORES = 8
N, C, H, W = 64, 256, 56, 56
HW = H * W                   # 3136
N_PER_CORE = N // N_CORES    # 8
B = 8                        # channel blocks
SIG = 16                     # spatial chunks per image
C_IN = 32                    # channels per block
S = HW // SIG                # 196
P = B * SIG                  # 128 partitions
TILES = N_PER_CORE           # 8 (one image per tile)
DMAS = B                     # dma_starts per tile per direction
INC = 16 * DMAS              # load-sem increment per tile (128)

TRACE = False
LAST_RESULTS = None
_CACHE = {}


def _build(mbits: int):
    import concourse.bass as bass
    from concourse import mybir

    nc = bass.Bass()
    x_in = nc.declare_dram_parameter(
        "x", [N_PER_CORE, C, HW], mybir.dt.float32, isOutput=False
    )
    q_out = nc.declare_dram_parameter(
        "q", [N_PER_CORE, C, HW], mybir.dt.bfloat16, isOutput=True
    )
    src = x_in[:].rearrange("n (b c) (g s) -> n b g c s", c=C_IN, s=S)
    dst = q_out[:].rearrange("n (b c) (g s) -> n b g c s", c=C_IN, s=S)

    i32, f32, i16, bf16 = (
        mybir.dt.int32, mybir.dt.float32, mybir.dt.int16, mybir.dt.bfloat16
    )
    Alu = mybir.AluOpType
    Act = mybir.ActivationFunctionType

    from contextlib import ExitStack
    es = ExitStack()
    with es:
        sb = lambda nm, shape, dt: es.enter_context(nc.sbuf_tensor(nm, shape, dt))
        X0 = sb("X0", [P, C_IN, S], f32); X1 = sb("X1", [P, C_IN, S], f32)
        YA0 = sb("YA0", [P, C_IN, S], f32); YA1 = sb("YA1", [P, C_IN, S], f32)
        T16 = sb("T16", [P, C_IN, S], i16)
        NU16 = sb("NU16", [P, C_IN, S], i16)
        QB0 = sb("QB0", [P, C_IN, S], bf16); QB1 = sb("QB1", [P, C_IN, S], bf16)
        SG0 = sb("SG0", [P, C_IN, S], bf16); SG1 = sb("SG1", [P, C_IN, S], bf16)
        M = sb("Mt", [P, S], f32)
        Pt = sb("Ptt", [P, S], i32)
        INV = sb("INVt", [P, S], f32)
        SC0 = sb("SC0", [P, S], bf16); SC1 = sb("SC1", [P, S], bf16)
        load_sem = es.enter_context(nc.semaphore())
        store_sem = es.enter_context(nc.semaphore())
        dve_sem = es.enter_context(nc.semaphore())
        act_sem = es.enter_context(nc.semaphore())
        block = es.enter_context(nc.Block())
        X = [X0, X1]
        YA = [YA0, YA1]
        QB = [QB0, QB1]
        SG = [SG0, SG1]
        SC = [SC0, SC1]
        y_done = {}    # dve counter after ya(t)
        tr_done = {}   # dve counter after tr(t)
        s_done = {}    # dve counter after final sign mult(t)

        def bc(ap):
            return ap.unsqueeze(1).broadcast_to((P, C_IN, S))

        @block.vector
        def _(vector):
            k = 0

            def step(inst):
                nonlocal k
                inst.then_inc(dve_sem, 1)
                k += 1
                vector.wait_ge(dve_sem, k)

            def front(t):
                nonlocal k
                xb = X[t % 2]
                vector.wait_ge(load_sem, INC * (t + 1))
                if t >= 2:
                    # YA[t%2] is free once ACT's nu16(t-2) is done
                    vector.wait_ge(act_sem, 4 * (t - 2) + 4)
                step(vector.tensor_reduce(
                    out=M[:], in_=xb[:].rearrange("p c s -> p s c"),
                    axis=mybir.AxisListType.X, op=Alu.max,
                    apply_absolute_value=True,
                ))
                step(vector.tensor_scalar(
                    out=Pt[:], in0=M[:].bitcast(i32),
                    scalar1=0x7F800000, scalar2=None, op0=Alu.bitwise_and,
                ))
                step(vector.tensor_scalar(
                    out=SC[t % 2][:], in0=Pt[:].bitcast(f32),
                    scalar1=float(2.0 ** (1 - mbits)), scalar2=float(2.0 ** -126),
                    op0=Alu.mult, op1=Alu.max,
                ))
                step(vector.tensor_scalar(
                    out=INV[:].bitcast(i32), in0=Pt[:],
                    scalar1=(253 + mbits) << 23, scalar2=-1,
                    op0=Alu.subtract, op1=Alu.mult,
                ))
                step(vector.tensor_tensor(
                    out=YA[t % 2][:], in0=xb[:], in1=bc(INV[:]), op=Alu.mult,
                ))
                y_done[t] = k

            def back(t):
                nonlocal k
                vector.wait_ge(act_sem, 4 * t + 4)  # ACT(t) fully done
                if t >= 2:
                    vector.wait_ge(store_sem, INC * (t - 1))  # QB[t%2] free
                # u16m1 = -nu16 - 1 (in-place)
                step(vector.tensor_scalar(
                    out=NU16[:], in0=NU16[:],
                    scalar1=-1, scalar2=-1, op0=Alu.mult, op1=Alu.add,
                ))
                # tr = max(u16m1, t16) (in-place into T16)
                step(vector.tensor_tensor(
                    out=T16[:], in0=NU16[:], in1=T16[:], op=Alu.max,
                ))
                tr_done[t] = k
                # qpos = tr * SC -> bf16
                step(vector.tensor_tensor(
                    out=QB[t % 2][:], in0=T16[:], in1=bc(SC[t % 2][:]),
                    op=Alu.mult,
                ))
                # q = qpos * sign(x)
                step(vector.tensor_tensor(
                    out=QB[t % 2][:], in0=QB[t % 2][:], in1=SG[t % 2][:],
                    op=Alu.mult,
                ))
                s_done[t] = k

            front(0)
            for t in range(1, TILES):
                front(t)
                back(t - 1)
            back(TILES - 1)

        @block.scalar
        def _(scalar):
            for t in range(TILES):
                # sgn(t) reads X[t%2], writes SG[t%2] (read by S(t-2) on DVE)
                if t >= 2:
                    scalar.wait_ge(dve_sem, s_done[t - 2])
                scalar.wait_ge(load_sem, INC * (t + 1))
                scalar.activation(
                    out=SG[t % 2][:], in_=X[t % 2][:],
                    func=Act.Sign, bias=0.0, scale=1.0,
                ).then_inc(act_sem, 1)
                # A = |ya| in-place; needs DVE ya(t)
                scalar.wait_ge(dve_sem, y_done[t])
                scalar.activation(
                    out=YA[t % 2][:], in_=YA[t % 2][:],
                    func=Act.Abs, bias=0.0, scale=1.0,
                ).then_inc(act_sem, 1)
                # t16/nu16 overwrite buffers consumed by DVE back(t-1)
                if t >= 1:
                    scalar.wait_ge(dve_sem, tr_done[t - 1])
                scalar.activation(
                    out=T16[:], in_=YA[t % 2][:],
                    func=Act.Copy, bias=-0.5, scale=1.0,
                ).then_inc(act_sem, 1)
                scalar.activation(
                    out=NU16[:], in_=YA[t % 2][:],
                    func=Act.Copy, bias=-0.5, scale=-1.0,
                ).then_inc(act_sem, 1)

        def issue_loads(sync, t):
            xb = X[t % 2]
            for b in range(B):
                sync.dma_start(
                    out=xb[b * SIG:(b + 1) * SIG], in_=src[t, b]
                ).then_inc(load_sem, 16)

        @block.sync
        def _(sync):
            issue_loads(sync, 0)
            sync.wait_ge(load_sem, INC)
            issue_loads(sync, 1)
            for t in range(TILES):
                if t + 2 < TILES:
                    # X[t%2] free once DVE's ya(t) and ACT's sgn(t) read it;
                    # prefetch loads(t+2) ahead of this tile's stores.
                    sync.wait_ge(act_sem, 4 * t + 1)
                    sync.wait_ge(dve_sem, y_done[t])
                    issue_loads(sync, t + 2)
                sync.wait_ge(dve_sem, s_done[t])
                qb = QB[t % 2]
                for b in range(B):
                    sync.dma_start(
                        out=dst[t, b], in_=qb[b * SIG:(b + 1) * SIG]
                    ).then_inc(store_sem, 16)

    return nc


def kernel(activations, mantissa_bits, blk, **_ignored):
    global LAST_RESULTS
    from concourse.bass_utils import run_bass_kernel_spmd

    mbits = int(mantissa_bits)
    assert int(blk) == C_IN, f"kernel hardcodes blk=32, got {blk}"
    assert 1 <= mbits <= 8, f"bf16 output path requires mantissa_bits<=8, got {mbits}"
    x = np.ascontiguousarray(np.asarray(activations), dtype=np.float32)
    assert x.shape == (N, C, H, W), x.shape

    if mbits not in _CACHE:
        _CACHE[mbits] = _build(mbits)
    nc = _CACHE[mbits]

    shards = x.reshape(N_CORES, N_PER_CORE, C, HW)
    in_maps = [{"x": shards[i]} for i in range(N_CORES)]
    res = run_bass_kernel_spmd(nc, in_maps, list(range(N_CORES)), trace=TRACE)
    LAST_RESULTS = res
    out = np.stack(
        [res.results[i]["q"].astype(np.float32) for i in range(N_CORES)], axis=0
    )
    return out.reshape(N, C, H, W)


# revision 9
# speedup vs baseline: 1.6135x; 1.0871x over previous
"""BFP activation quantization kernel for Trainium2 (8 NeuronCores).

Problem: x (64, 256, 56, 56) fp32. Channels grouped in blocks of 32; each
block shares the max frexp-exponent emax; mantissas truncated to
`mantissa_bits` bits relative to 2^emax:
    q = trunc(x / 2^(emax-mb)) * 2^(emax-mb)

Pipeline (all identities bit-exact, verified on hardware):
  - M = max_c |x| per (block, pixel); Pt = bits(M) & 0x7F800000 = 2^(emax-1).
  - SC = 2^(emax-mb) (kept in bf16: exact, power of two); INV = 2^(mb-emax)
    built by integer exponent arithmetic on Pt (exact reciprocal).
  - ya = x * INV (signed; exact power-of-two scaling), |ya| in [0, 2^mb).
  - ACT engine: A = |ya|; the fp32->int16 convert rounds to nearest-even, so
    trunc is built from two rounded candidates:
        t16  = rne16(A - 0.5)        (ACT, bias=-0.5)
        nu16 = rne16(-A - 0.5) = -u16(ACT, scale=-1, bias=-0.5)
        u16m1 = -nu16 - 1            (DVE int16 ts: mult -1, add -1)
        tr   = max(u16m1, t16)       (DVE int16 TT)
    which equals trunc(|ya|) for every case incl. integer ya and half ties.
  - q = (tr * SC) * Sign(x): tr <= 2^mb - 1 has <= 8 significant bits, so for
    mb <= 8 every product is exactly representable in bf16; Sign(x) comes
    from the ACT engine as bf16 +-1 (0 for x == 0, where q = 0 anyway).
  - Output is stored as bf16 (exact) and widened to fp32 on the host.

Engine split per tile (1 image; partition p = (b<8, g<16), free = (c32,s196)):
  DVE: reduce + 3 small [P,196] ts + ya-mult + int16/bf16 tail (4 ops)
  ACT: Sign, Abs, t16, nu16 (own port; overlaps DVE fully)
  SP:  8 load + 8 store DMAs per tile, double-buffered
(GPSIMD/Pool cannot run TensorTensor/TensorScalar in this toolchain; PE's
fp32r matmuls are not bit-exact - both verified - so DVE+ACT is the split.)

Sharding: data-parallel on N across 8 cores, no cross-core communication.
"""

import numpy as np

N_CORES = 8
N, C, H, W = 64, 256, 56, 56
HW = H * W                   # 3136
N_PER_CORE = N // N_CORES    # 8
B = 8                        # channel blocks
SIG = 16                     # spatial chunks per image
C_IN = 32                    # channels per block
S = HW // SIG                # 196
P = B * SIG                  # 128 partitions
TILES = N_PER_CORE           # 8 (one image per tile)
DMAS = B                     # dma_starts per tile per direction
INC = 16 * DMAS              # load-sem increment per tile (128)

TRACE = False
LAST_RESULTS = None
_CACHE = {}


def _build(mbits: int):
    import concourse.bass as bass
    from concourse import mybir

    nc = bass.Bass()
    x_in = nc.declare_dram_parameter(
        "x", [N_PER_CORE, C, HW], mybir.dt.float32, isOutput=False
    )
    q_out = nc.declare_dram_parameter(
        "q", [N_PER_CORE, C, HW], mybir.dt.bfloat16, isOutput=True
    )
    src = x_in[:].rearrange("n (b c) (g s) -> n b g c s", c=C_IN, s=S)
    dst = q_out[:].rearrange("n (b c) (g s) -> n b g c s", c=C_IN, s=S)

    i32, f32, i16, bf16 = (
        mybir.dt.int32, mybir.dt.float32, mybir.dt.int16, mybir.dt.bfloat16
    )
    Alu = mybir.AluOpType
    Act = mybir.ActivationFunctionType

    from contextlib import ExitStack
    es = ExitStack()
    with es:
        sb = lambda nm, shape, dt: es.enter_context(nc.sbuf_tensor(nm, shape, dt))
        X0 = sb("X0", [P, C_IN, S], f32); X1 = sb("X1", [P, C_IN, S], f32)
        YA0 = sb("YA0", [P, C_IN, S], f32); YA1 = sb("YA1", [P, C_IN, S], f32)
        T16 = sb("T16", [P, C_IN, S], i16)
        NU16 = sb("NU16", [P, C_IN, S], i16)
        QB0 = sb("QB0", [P, C_IN, S], bf16); QB1 = sb("QB1", [P, C_IN, S], bf16)
        QB2 = sb("QB2", [P, C_IN, S], bf16)
        SG0 = sb("SG0", [P, C_IN, S], bf16); SG1 = sb("SG1", [P, C_IN, S], bf16)
        M = sb("Mt", [P, S], f32)
        Pt = sb("Ptt", [P, S], i32)
        INV = sb("INVt", [P, S], f32)
        SC0 = sb("SC0", [P, S], bf16); SC1 = sb("SC1", [P, S], bf16)
        load_sem = es.enter_context(nc.semaphore())
        store_sem = es.enter_context(nc.semaphore())
        dve_sem = es.enter_context(nc.semaphore())
        act_sem = es.enter_context(nc.semaphore())
        act7_sem = es.enter_context(nc.semaphore())
        block = es.enter_context(nc.Block())
        X = [X0, X1]
        YA = [YA0, YA1]
        QB = [QB0, QB1, QB2]
        SG = [SG0, SG1]
        SC = [SC0, SC1]
        y_done = {}    # dve counter after ya(t)
        tr_done = {}   # dve counter after tr(t)
        s_done = {}    # dve counter after final sign mult(t)

        def bc(ap):
            return ap.unsqueeze(1).broadcast_to((P, C_IN, S))

        @block.vector
        def _(vector):
            k = 0

            def step(inst):
                # same-engine ordering is guaranteed by the in-order queue +
                # pipeline drain; the inc is only for cross-engine consumers.
                nonlocal k
                inst.then_inc(dve_sem, 1)
                k += 1

            def front(t):
                nonlocal k
                xb = X[t % 2]
                vector.wait_ge(load_sem, INC * (t + 1))
                if t >= 2:
                    # YA[t%2] is free once ACT's nu16(t-2) is done
                    vector.wait_ge(act_sem, 4 * (t - 2) + 4)
                step(vector.tensor_reduce(
                    out=M[:], in_=xb[:].rearrange("p c s -> p s c"),
                    axis=mybir.AxisListType.X, op=Alu.max,
                    apply_absolute_value=True,
                ))
                step(vector.tensor_scalar(
                    out=Pt[:], in0=M[:].bitcast(i32),
                    scalar1=0x7F800000, scalar2=None, op0=Alu.bitwise_and,
                ))
                step(vector.tensor_scalar(
                    out=SC[t % 2][:], in0=Pt[:].bitcast(f32),
                    scalar1=float(2.0 ** (1 - mbits)), scalar2=float(2.0 ** -126),
                    op0=Alu.mult, op1=Alu.max,
                ))
                step(vector.tensor_scalar(
                    out=INV[:].bitcast(i32), in0=Pt[:],
                    scalar1=(253 + mbits) << 23, scalar2=-1,
                    op0=Alu.subtract, op1=Alu.mult,
                ))
                step(vector.tensor_tensor(
                    out=YA[t % 2][:], in0=xb[:], in1=bc(INV[:]), op=Alu.mult,
                ))
                y_done[t] = k

            def back(t):
                nonlocal k
                vector.wait_ge(act_sem, 4 * t + 3)  # nu16(t) done
                if t >= 3:
                    vector.wait_ge(store_sem, INC * (t - 2))  # QB[t%3] free
                # u16m1 = -nu16 - 1 (in-place)
                step(vector.tensor_scalar(
                    out=NU16[:], in0=NU16[:],
                    scalar1=-1, scalar2=-1, op0=Alu.mult, op1=Alu.add,
                ))
                # tr = max(u16m1, t16) (in-place into T16)
                vector.wait_ge(act_sem, 4 * t + 4)  # t16(t) done
                step(vector.tensor_tensor(
                    out=T16[:], in0=NU16[:], in1=T16[:], op=Alu.max,
                ))
                tr_done[t] = k
                # qpos = tr * SC -> bf16
                step(vector.tensor_tensor(
                    out=QB[t % 3][:], in0=T16[:], in1=bc(SC[t % 2][:]),
                    op=Alu.mult,
                ))
                # q = qpos * sign(x)
                step(vector.tensor_tensor(
                    out=QB[t % 3][:], in0=QB[t % 3][:], in1=SG[t % 2][:],
                    op=Alu.mult,
                ))
                s_done[t] = k

            def back7(half):
                nonlocal k
                lo, hi = (0, S // 2) if half == 0 else (S // 2, S)
                sl = slice(lo, hi)
                vector.wait_ge(act7_sem, 2 + 3 * half)  # nu16(7,half)
                if half == 0:
                    vector.wait_ge(store_sem, INC * 5)
                step(vector.tensor_scalar(
                    out=NU16[:, :, sl], in0=NU16[:, :, sl],
                    scalar1=-1, scalar2=-1, op0=Alu.mult, op1=Alu.add,
                ))
                vector.wait_ge(act7_sem, 3 + 3 * half)  # t16(7,half)
                step(vector.tensor_tensor(
                    out=T16[:, :, sl], in0=NU16[:, :, sl], in1=T16[:, :, sl],
                    op=Alu.max,
                ))
                step(vector.tensor_tensor(
                    out=QB[(TILES - 1) % 3][:, :, sl], in0=T16[:, :, sl],
                    in1=SC[(TILES - 1) % 2][:, sl].unsqueeze(1).broadcast_to(
                        (P, C_IN, hi - lo)),
                    op=Alu.mult,
                ))
                if half == 0:
                    vector.wait_ge(act_sem, 4 * (TILES - 1) + 1)  # sgn(7)
                step(vector.tensor_tensor(
                    out=QB[(TILES - 1) % 3][:, :, sl], in0=QB[(TILES - 1) % 3][:, :, sl],
                    in1=SG[(TILES - 1) % 2][:, :, sl], op=Alu.mult,
                ))
                s_done[(TILES - 1, half)] = k

            front(0)
            for t in range(1, TILES):
                front(t)
                if t - 1 == TILES - 1:
                    break
                back(t - 1)
            back7(0)
            back7(1)

        @block.scalar
        def _(scalar):
            for t in range(TILES - 1):
                # sgn(t) reads X[t%2], writes SG[t%2] (read by S(t-2) on DVE)
                if t >= 2:
                    scalar.wait_ge(dve_sem, s_done[t - 2])
                scalar.wait_ge(load_sem, INC * (t + 1))
                scalar.activation(
                    out=SG[t % 2][:], in_=X[t % 2][:],
                    func=Act.Sign, bias=0.0, scale=1.0,
                ).then_inc(act_sem, 1)
                # A = |ya| in-place; needs DVE ya(t)
                scalar.wait_ge(dve_sem, y_done[t])
                scalar.activation(
                    out=YA[t % 2][:], in_=YA[t % 2][:],
                    func=Act.Abs, bias=0.0, scale=1.0,
                ).then_inc(act_sem, 1)
                # nu16/t16 overwrite buffers consumed by DVE back(t-1);
                # nu16 first so DVE's u16m1(t) can start one ACT op earlier
                if t >= 1:
                    scalar.wait_ge(dve_sem, tr_done[t - 1])
                scalar.activation(
                    out=NU16[:], in_=YA[t % 2][:],
                    func=Act.Copy, bias=-0.5, scale=-1.0,
                ).then_inc(act_sem, 1)
                scalar.activation(
                    out=T16[:], in_=YA[t % 2][:],
                    func=Act.Copy, bias=-0.5, scale=1.0,
                ).then_inc(act_sem, 1)
            # tile 7: halves, signalled on act7_sem
            t = TILES - 1
            scalar.wait_ge(dve_sem, s_done[t - 2])
            scalar.wait_ge(load_sem, INC * (t + 1))
            scalar.activation(
                out=SG[t % 2][:], in_=X[t % 2][:],
                func=Act.Sign, bias=0.0, scale=1.0,
            ).then_inc(act_sem, 1)
            scalar.wait_ge(dve_sem, y_done[t])
            scalar.wait_ge(dve_sem, tr_done[t - 1])
            for half in range(2):
                sl = slice(0, S // 2) if half == 0 else slice(S // 2, S)
                scalar.activation(
                    out=YA[t % 2][:, :, sl], in_=YA[t % 2][:, :, sl],
                    func=Act.Abs, bias=0.0, scale=1.0,
                ).then_inc(act7_sem, 1)
                scalar.activation(
                    out=NU16[:, :, sl], in_=YA[t % 2][:, :, sl],
                    func=Act.Copy, bias=-0.5, scale=-1.0,
                ).then_inc(act7_sem, 1)
                scalar.activation(
                    out=T16[:, :, sl], in_=YA[t % 2][:, :, sl],
                    func=Act.Copy, bias=-0.5, scale=1.0,
                ).then_inc(act7_sem, 1)

        def issue_loads(sync, t):
            xb = X[t % 2]
            for b in range(B):
                sync.dma_start(
                    out=xb[b * SIG:(b + 1) * SIG], in_=src[t, b]
                ).then_inc(load_sem, 16)

        @block.sync
        def _(sync):
            issue_loads(sync, 0)
            sync.wait_ge(load_sem, INC)
            issue_loads(sync, 1)
            for t in range(TILES):
                if t + 2 < TILES:
                    # X[t%2] free once DVE's ya(t) and ACT's sgn(t) read it;
                    # prefetch loads(t+2) ahead of this tile's stores.
                    sync.wait_ge(act_sem, 4 * t + 1)
                    sync.wait_ge(dve_sem, y_done[t])
                    issue_loads(sync, t + 2)
                if t < TILES - 1:
                    sync.wait_ge(dve_sem, s_done[t])
                    qb = QB[t % 3]
                    for b in range(B):
                        sync.dma_start(
                            out=dst[t, b], in_=qb[b * SIG:(b + 1) * SIG]
                        ).then_inc(store_sem, 16)
                else:
                    qb = QB[t % 3]
                    for half in range(2):
                        sl = slice(0, S // 2) if half == 0 else slice(S // 2, S)
                        sync.wait_ge(dve_sem, s_done[(t, half)])
                        for b in range(B):
                            sync.dma_start(
                                out=dst[t, b][:, :, sl],
                                in_=qb[b * SIG:(b + 1) * SIG][:, :, sl],
                            ).then_inc(store_sem, 16)

    return nc


def kernel(activations, mantissa_bits, blk, **_ignored):
    global LAST_RESULTS
    from concourse.bass_utils import run_bass_kernel_spmd

    mbits = int(mantissa_bits)
    assert int(blk) == C_IN, f"kernel hardcodes blk=32, got {blk}"
    assert 1 <= mbits <= 8, f"bf16 output path requires mantissa_bits<=8, got {mbits}"
    x = np.ascontiguousarray(np.asarray(activations), dtype=np.float32)
    assert x.shape == (N, C, H, W), x.shape

    if mbits not in _CACHE:
        _CACHE[mbits] = _build(mbits)
    nc = _CACHE[mbits]

    shards = x.reshape(N_CORES, N_PER_CORE, C, HW)
    in_maps = [{"x": shards[i]} for i in range(N_CORES)]
    res = run_bass_kernel_spmd(nc, in_maps, list(range(N_CORES)), trace=TRACE)
    LAST_RESULTS = res
    out = np.stack(
        [res.results[i]["q"].astype(np.float32) for i in range(N_CORES)], axis=0
    )
    return out.reshape(N, C, H, W)
